# revision 1
# baseline (speedup 1.0000x reference)
"""BehaviorMoE Trainium2 kernel (8 NeuronCores, SPMD data-parallel over sorted tokens).

Contract: kernel(**inputs) takes FULL inputs as returned by setup_inputs() and
returns the FULL [8192, 1024] float32 output.

Strategy:
  - Host: sort tokens by behavior id. Tokens with b==0 need no expert compute
    (output = x + beta); they are used as masked filler so that every core gets
    exactly 1024 tokens that share a single behavior id.  Each core receives
    the stacked weight matrix [W_sh0; W_sh1; W_sh2; W_sp[t]]^T for its behavior.
  - Device (identical SPMD program, per-core data):
      Phase B (gates): per 128-token tile, gate logits (PE), masked softmax
        (DVE/ACT), PE transpose of gates, bias combine via gates^T @ b_all (PE)
        copied into an SBUF accumulator.
      Phase C (experts): e-outer loop streams the stacked weights once while
        the PE runs a dense fp32r matmul stream; a fused DVE
        scalar_tensor_tensor accumulates gate-weighted expert outputs into
        ping-pong SBUF accumulators (in-place DVE ops fault on this HW).
      Phase D (tail): LayerNorm stats (ACT Square batched to avoid act-table
        reloads), normalize + residual, DMA out.
  - Host: scatter per-core outputs back to original token order.
"""

import os
import sys

import numpy as np

for _p in ("/opt/trn_rl_repo", "/root/.axon_site/_ro/trn_rl_repo"):
    if os.path.isdir(_p) and _p not in sys.path:
        sys.path.append(_p)

from contextlib import ExitStack

from concourse import bacc, bass, masks, mybir, tile
from concourse.bass_utils import run_bass_kernel_spmd

F32 = mybir.dt.float32
F32R = mybir.dt.float32r
AX = mybir.AxisListType
ALU = mybir.AluOpType
ACTF = mybir.ActivationFunctionType

D = 1024            # model dim
N = 8192            # tokens
NB = 4              # behaviors
NESH = 3            # shared experts
NE = 4              # experts per behavior (3 shared + 1 specific)
EPS = 1e-5
NCORES = 8
M = N // NCORES     # tokens per core
KT = D // 128       # k tiles (contraction)
IT = M // 128       # token tiles per core
FH = 512            # feature half-tile (psum bank width in f32)


def _build_program(trivial_affine: bool) -> bass.Bass:
    nc = bacc.Bacc()

    xt_d = nc.declare_dram_parameter("xt", [KT, 128, M], F32R, isOutput=False)
    xtok_d = nc.declare_dram_parameter("xtok", [M, D], F32, isOutput=False)
    wt_d = nc.declare_dram_parameter("wt", [NE, 2, KT, 128, FH], F32R, isOutput=False)
    wg_d = nc.declare_dram_parameter("wg", [128, KT * 128], F32R, isOutput=False)
    ball_d = nc.declare_dram_parameter("ball", [128, D], F32R, isOutput=False)
    mask_d = nc.declare_dram_parameter("mask", [128, IT], F32, isOutput=False)
    if not trivial_affine:
        gam_d = nc.declare_dram_parameter("gam", [128, D], F32, isOutput=False)
        bet_d = nc.declare_dram_parameter("bet", [128, D], F32, isOutput=False)
    out_d = nc.declare_dram_parameter("out", [M, D], F32, isOutput=True)

    with tile.TileContext(nc) as tc, ExitStack() as ctx:
        const = ctx.enter_context(tc.tile_pool(name="const", bufs=1))
        xtp = ctx.enter_context(tc.tile_pool(name="xt", bufs=KT))
        wpool = ctx.enter_context(tc.tile_pool(name="w", bufs=16))
        selp = ctx.enter_context(tc.tile_pool(name="sel", bufs=2 * IT))
        xtokp = ctx.enter_context(tc.tile_pool(name="xtok", bufs=3))
        outp = ctx.enter_context(tc.tile_pool(name="outp", bufs=2))
        scrp = ctx.enter_context(tc.tile_pool(name="scr", bufs=3))
        gatep = ctx.enter_context(tc.tile_pool(name="gate", bufs=IT))
        smallp = ctx.enter_context(tc.tile_pool(name="small", bufs=40))
        zpool = ctx.enter_context(tc.tile_pool(name="z", bufs=5, space="PSUM"))
        pspool = ctx.enter_context(tc.tile_pool(name="ps", bufs=3, space="PSUM"))

        # ---- PE warm-up source ----
        zsrc0 = const.tile([128, FH], F32, tag="zsrc0")
        nc.gpsimd.memset(zsrc0[:], 0.0)
        zsrc = const.tile([128, FH], F32R, tag="zsrc")
        nc.vector.tensor_copy(zsrc[:], zsrc0[:])

        dummy_state = {"n": 0}

        def dummies(n):
            """n PE filler matmuls (zero @ zero) keeping the PE array busy."""
            dt_ = pspool.tile([128, FH], F32, tag="ps", name=f"dps{dummy_state['n']}")
            dummy_state["n"] += 1
            for j in range(n):
                nc.tensor.matmul(
                    dt_[:], zsrc[:, 0:128], zsrc[:],
                    start=(j == 0), stop=(j == n - 1),
                )

        # ---- small constant inputs ----
        wg_sb = const.tile([128, KT * 128], F32R, tag="wg")
        nc.sync.dma_start(wg_sb[:], wg_d[:])
        ball_sb = const.tile([128, D], F32R, tag="ball")
        nc.sync.dma_start(ball_sb[:], ball_d[:])
        mask_sb = const.tile([128, IT], F32, tag="mask")
        nc.sync.dma_start(mask_sb[:], mask_d[:])
        if not trivial_affine:
            gam_sb = const.tile([128, D], F32, tag="gam")
            nc.sync.dma_start(gam_sb[:], gam_d[:])
            bet_sb = const.tile([128, D], F32, tag="bet")
            nc.sync.dma_start(bet_sb[:], bet_d[:])

        # ---- resident xT k-tiles first, then streamed weight half-tiles ----
        xT = []
        for k in range(KT):
            t = xtp.tile([128, M], F32R, tag="xt")
            nc.sync.dma_start(t[:], xt_d[k])
            xT.append(t)
        w_sb = {}
        for e in range(NE):
            for c in (0, 1):
                for k in range(KT):
                    t = wpool.tile([128, FH], F32R, tag="w", name=f"w{e}{c}{k}")
                    nc.sync.dma_start(t[:], wt_d[e, c, k])
                    w_sb[(e, c, k)] = t

        identity = const.tile([128, 128], F32, tag="ident")
        masks.make_identity(nc, identity[:])
        identR = const.tile([128, 128], F32R, tag="identR")
        nc.vector.tensor_copy(identR[:], identity[:])

        # ---- accumulators (ping-pong; in-place DVE ops fault) ----
        selA = [selp.tile([128, D], F32, tag="sel", name=f"selA{i}") for i in range(IT)]
        selB = [selp.tile([128, D], F32, tag="sel", name=f"selB{i}") for i in range(IT)]

        # ---- gate logits glT[4, tok], k-paced by the xT DMAs; dummy
        #      matmuls keep the PE dense so the HAM never sees sparse work ----
        dummies(8)
        glT_ps = {}
        for c in (0, 1):
            glT_ps[c] = pspool.tile([128, FH], F32, tag="ps", name=f"glTps{c}")
        for k in range(KT):
            for c in (0, 1):
                nc.tensor.matmul(
                    glT_ps[c][:], wg_sb[:, k * 128:(k + 1) * 128],
                    xT[k][:, c * FH:(c + 1) * FH],
                    start=(k == 0), stop=(k == KT - 1),
                )
            if k < KT - 1:
                dummies(5)
        glT_sb = const.tile([NE, M], F32R, tag="glT")
        for c in (0, 1):
            nc.vector.tensor_copy(glT_sb[:, c * FH:(c + 1) * FH], glT_ps[c][0:NE, :])

        # ---- per token tile: logits transpose (as a plain matmul so the
        #      PE array stays active), masked softmax, gates transpose,
        #      bias-combine matmuls into the accumulator ----
        glp_t = []
        for i in range(IT):
            glp = pspool.tile([128, NE], F32, tag="ps", name=f"glp{i}")
            nc.tensor.matmul(
                glp[:], glT_sb[:, i * 128:(i + 1) * 128], identR[0:NE, 0:NE],
                start=True, stop=True,
            )
            glp_t.append(glp)
            dummies(2)
        gates_t = []
        for i in range(IT):
            glp = glp_t[i]
            negmax = smallp.tile([128, 1], F32, tag="s1")
            nc.vector.tensor_reduce(
                negmax[:], glp[:], axis=AX.X, op=ALU.max, negate=True
            )
            exps = smallp.tile([128, NE], F32, tag="s4")
            expsum = smallp.tile([128, 1], F32, tag="s1")
            nc.scalar.activation(
                exps[:], glp[:], ACTF.Exp,
                bias=negmax[:], scale=1.0, accum_out=expsum[:],
            )
            rinv = smallp.tile([128, 1], F32, tag="s1")
            nc.vector.reciprocal(rinv[:], expsum[:])
            rm = smallp.tile([128, 1], F32, tag="s1")
            nc.vector.tensor_mul(rm[:], rinv[:], mask_sb[:, i:i + 1])
            gates = gatep.tile([128, NE], F32R, tag="g")
            nc.vector.tensor_scalar_mul(gates[:], exps[:], rm[:])
            gates_t.append(gates)
        dummies(6)
        gTp = const.tile([128, 128], F32R, tag="gTp")
        nc.vector.tensor_copy(gTp[:], zsrc0[:, 0:128])  # rows 4+ stay zero
        for i in range(IT):
            gtp = pspool.tile([NE, 128], F32, tag="ps", name=f"gtp{i}")
            nc.tensor.matmul(
                gtp[:], gates_t[i][:], identR[:], start=True, stop=True
            )
            nc.vector.tensor_copy(gTp[0:NE, :], gtp[:])
            for c in (0, 1):
                bp = zpool.tile([128, FH], F32, tag="z", name=f"bps{i}{c}")
                nc.tensor.matmul(
                    bp[:], gTp[:], ball_sb[:, c * FH:(c + 1) * FH],
                    start=True, stop=True,
                )
                nc.scalar.copy(selA[i][:, c * FH:(c + 1) * FH], bp[:])
            dummies(2)
        dummies(6)

        # ---- expert matmul stream: half-pass (c) outer so each half-pass
        #      only needs 2.1MB of fresh weights -> stall-free stream start.
        #      LN tail split across the two e3 half-passes. ----
        bn6s = [None] * IT
        src_l, dst_l = selA, selB
        for e in range(NE):
            last = e == NE - 1
            for c in (0, 1):
                cs = slice(c * FH, (c + 1) * FH)
                for i in range(IT):
                    isl = slice(i * 128, (i + 1) * 128)
                    zt = zpool.tile([128, FH], F32, tag="z")
                    for k in range(KT):
                        nc.tensor.matmul(
                            zt[:], xT[k][:, isl], w_sb[(e, c, k)][:],
                            start=(k == 0), stop=(k == KT - 1),
                        )
                    if not last:
                        nc.vector.scalar_tensor_tensor(
                            dst_l[i][:, cs], zt[:], gates_t[i][:, e:e + 1],
                            src_l[i][:, cs], op0=ALU.mult, op1=ALU.add,
                        )
                        continue
                    nc.vector.scalar_tensor_tensor(
                        dst_l[i][:, cs], zt[:], gates_t[i][:, e:e + 1],
                        src_l[i][:, cs], op0=ALU.mult, op1=ALU.add,
                    )
                    if c == 0:
                        bn6 = smallp.tile([128, 2 * 6], F32, tag="bn6")
                        nc.vector.bn_stats(bn6[:, 0:6], dst_l[i][:, 0:FH])
                        bn6s[i] = bn6
                        continue
                    selF = dst_l[i]
                    bn6 = bn6s[i]
                    nc.vector.bn_stats(bn6[:, 6:12], selF[:, FH:D])
                    mv = smallp.tile([128, 2], F32, tag="mv")
                    nc.vector.bn_aggr(mv[:], bn6[:])
                    avi = smallp.tile([128, 1], F32, tag="s1")
                    nc.vector.tensor_scalar_add(avi[:], mv[:, 1:2], EPS)
                    sdi = smallp.tile([128, 1], F32, tag="s1")
                    nc.scalar.sqrt(sdi[:], avi[:])
                    ri = smallp.tile([128, 1], F32, tag="s1")
                    nc.vector.reciprocal(ri[:], sdi[:])
                    mbt = smallp.tile([128, 1], F32, tag="s1")
                    nc.vector.tensor_mul(mbt[:], mv[:, 0:1], ri[:])
                    mbi = smallp.tile([128, 1], F32, tag="s1")
                    nc.vector.tensor_scalar_mul(mbi[:], mbt[:], -1.0)
                    # ln = sel*rstd + mb on ACT, residual adds on GpSimd
                    xi = xtokp.tile([128, D], F32, tag="xtok")
                    nc.sync.dma_start(xi[:], xtok_d[i * 128:(i + 1) * 128, :])
                    lnb = scrp.tile([128, D], F32, tag="scr")
                    nc.scalar.activation(
                        lnb[:], selF[:], ACTF.Identity, bias=mbi[:], scale=ri[:]
                    )
                    if not trivial_affine:
                        lng = scrp.tile([128, D], F32, tag="scr")
                        nc.vector.tensor_mul(lng[:], lnb[:], gam_sb[:])
                        lnb2 = scrp.tile([128, D], F32, tag="scr")
                        nc.vector.tensor_add(lnb2[:], lng[:], bet_sb[:])
                        lnb = lnb2
                    outt = outp.tile([128, D], F32, tag="out")
                    nc.gpsimd.tensor_add(outt[:, 0:FH], lnb[:, 0:FH], xi[:, 0:FH])
                    nc.gpsimd.tensor_add(outt[:, FH:D], lnb[:, FH:D], xi[:, FH:D])
                    nc.sync.dma_start(out_d[i * 128:(i + 1) * 128, :], outt[:])
            src_l, dst_l = dst_l, src_l

    nc.finalize()
    return nc


_PROGRAM_CACHE: dict = {}


def _get_program(trivial_affine: bool) -> bass.Bass:
    key = trivial_affine
    if key not in _PROGRAM_CACHE:
        _PROGRAM_CACHE[key] = _build_program(trivial_affine)
    return _PROGRAM_CACHE[key]


def _pack_tokens(b: np.ndarray):
    """Partition 8192 tokens into 8 chunks of 1024, each chunk holding tokens
    of a single behavior (1..4) plus masked b==0 filler."""
    idx0 = np.flatnonzero(b == 0)
    chunks = []
    for t in range(1, NB + 1):
        idxs = np.flatnonzero(b == t)
        for s in range(0, max(len(idxs), 1), M):
            part = idxs[s:s + M]
            if len(part) or not chunks:
                chunks.append((part, t))
    chunks = [(p, t) for (p, t) in chunks if len(p) > 0]
    if len(chunks) > NCORES:
        raise RuntimeError(
            f"token packing needs {len(chunks)} single-behavior chunks > {NCORES}"
        )
    while len(chunks) < NCORES:
        chunks.append((np.empty((0,), np.int64), 1))
    p0 = 0
    cores = []
    for part, t in chunks:
        need = M - len(part)
        fill = idx0[p0:p0 + need]
        p0 += need
        if len(fill) != need:
            raise RuntimeError("not enough b==0 filler tokens for packing")
        idx = np.concatenate([part.astype(np.int64), fill.astype(np.int64)])
        msk = np.zeros((M,), np.float32)
        msk[:len(part)] = 1.0
        cores.append((idx, msk, t))
    assert p0 == len(idx0)
    return cores


def _behavior_tensors(W_sh, b_sh, W_sp, b_sp, w_gates):
    per_t = {}
    W_sh_flat = W_sh.reshape(NESH * D, D)
    for t in range(1, NB + 1):
        Wall = np.concatenate([W_sh_flat, W_sp[t - 1:t].reshape(D, D)], axis=0)
        wT = np.ascontiguousarray(Wall.T)                      # [D, NE*D]
        wt_h = np.ascontiguousarray(
            wT.reshape(KT, 128, NE, 2, FH).transpose(2, 3, 0, 1, 4)
        )                                                      # [e, c, k, 128, FH]
        wg_h = np.zeros((128, KT * 128), np.float32)
        wg_k = w_gates[t - 1].reshape(KT, 128, NE).transpose(1, 0, 2)  # [128, KT, NE]
        for k in range(KT):
            wg_h[:, k * 128:k * 128 + NE] = wg_k[:, k, :]
        ball_h = np.zeros((128, D), np.float32)
        ball_h[0:NE] = np.stack([b_sh[0], b_sh[1], b_sh[2], b_sp[t - 1]], axis=0)
        per_t[t] = (wt_h, wg_h, ball_h)
    return per_t


def _prepare(x, b_seq, W_sh, b_sh, W_sp, b_sp, w_gates, gamma, beta):
    x = np.ascontiguousarray(np.asarray(x, dtype=np.float32))
    b = np.asarray(b_seq).astype(np.int64).ravel()
    W_sh = np.asarray(W_sh, dtype=np.float32)
    b_sh = np.asarray(b_sh, dtype=np.float32)
    W_sp = np.asarray(W_sp, dtype=np.float32)
    b_sp = np.asarray(b_sp, dtype=np.float32)
    w_gates = np.asarray(w_gates, dtype=np.float32)
    gamma = np.asarray(gamma, dtype=np.float32)
    beta = np.asarray(beta, dtype=np.float32)
    assert x.shape == (N, D) and b.shape == (N,)

    trivial = bool(np.all(gamma == 1.0) and np.all(beta == 0.0))
    cores = _pack_tokens(b)
    per_t = _behavior_tensors(W_sh, b_sh, W_sp, b_sp, w_gates)

    in_maps = []
    for idx, msk, t in cores:
        wt_h, wg_h, ball_h = per_t[t]
        xc = np.ascontiguousarray(x[idx])                      # [M, D]
        xt_h = np.ascontiguousarray(xc.T).reshape(KT, 128, M)  # [k, 128, M]
        m = {
            "xt": xt_h,
            "xtok": xc,
            "wt": wt_h,
            "wg": wg_h,
            "ball": ball_h,
            "mask": np.ascontiguousarray(msk.reshape(IT, 128).T),
        }
        if not trivial:
            m["gam"] = np.ascontiguousarray(np.broadcast_to(gamma, (128, D)))
            m["bet"] = np.ascontiguousarray(np.broadcast_to(beta, (128, D)))
        in_maps.append(m)
    return trivial, cores, in_maps


def kernel_with_results(trace: bool = False, **inputs):
    trivial, cores, in_maps = _prepare(**inputs)
    nc = _get_program(trivial)
    res = run_bass_kernel_spmd(
        nc, in_maps, list(range(NCORES)), trace=trace
    )
    out = np.empty((N, D), np.float32)
    for c, (idx, _msk, _t) in enumerate(cores):
        out[idx] = res.results[c]["out"]
    return out, res


def kernel(**inputs) -> np.ndarray:
    out, _ = kernel_with_results(trace=False, **inputs)
    return out



# revision 4
# speedup vs baseline: 1.2686x; 1.2686x over previous
"""BehaviorMoE Trainium2 kernel (8 NeuronCores, SPMD data-parallel over sorted tokens).

Contract: kernel(**inputs) takes FULL inputs as returned by setup_inputs() and
returns the FULL [8192, 1024] float32 output.

v3 strategy (vs v0 baseline at ~197us):
  - Host: sort tokens by behavior id into 8 single-behavior chunks of M=896
    (7x128 tiles). b==0 tokens need no expert compute; a few are used as
    masked filler inside partial tiles (their device output x+beta is still
    correct), the rest get out = x + beta on host. This drops PE work 12.5%
    vs the 1024-token packing.
  - Weights and the expert-side copy of x are bf16: halves weight DMA
    (16.8MB -> 8.4MB f32-equiv) while the PE runs bf16 at the same
    1 cycle/row as f32r. Gate logits stay f32r (softmax amplifies absolute
    logit error, bf16 there is not acceptable).
  - Device: phase B (gates) paced by the 3.6MB f32 x DMA, with light bf16
    dummy matmuls only; expert stream e-outer/c-half/i-inner with ping-pong
    SBUF accumulators via DVE scalar_tensor_tensor.
  - Tail: LN + residual per tile is split DVE/ACT/GpSimd and overlapped with
    the e3 matmul stream; xtok prefetched one c-pass early; single act-table
    switch (Exp table in phase B -> Rsqrt table in LN).
"""

import os
import sys

import numpy as np
import ml_dtypes

for _p in ("/opt/trn_rl_repo", "/root/.axon_site/_ro/trn_rl_repo"):
    if os.path.isdir(_p) and _p not in sys.path:
        sys.path.append(_p)

from contextlib import ExitStack

from concourse import bacc, bass, masks, mybir, tile
from concourse.bass_utils import run_bass_kernel_spmd

F32 = mybir.dt.float32
F32R = mybir.dt.float32r
BF16 = mybir.dt.bfloat16
AX = mybir.AxisListType
ALU = mybir.AluOpType
ACTF = mybir.ActivationFunctionType

D = 1024            # model dim
N = 8192            # tokens
NB = 4              # behaviors
NESH = 3            # shared experts
NE = 4              # experts per behavior (3 shared + 1 specific)
EPS = 1e-5
NCORES = 8
KT = D // 128       # k tiles (contraction)
FH = 512            # feature half-tile (psum bank width in f32)
M_FULL = N // NCORES       # 1024: fallback packing (all tokens placed)
M_SKIP = 896               # 7 tiles: b==0 tokens mostly skipped

BF = ml_dtypes.bfloat16


def _build_program(m_tok: int, trivial_affine: bool) -> bass.Bass:
    IT = m_tok // 128
    H = m_tok // 2          # token half width for gate-logit psum
    nc = bacc.Bacc()

    xt_d = nc.declare_dram_parameter("xt", [KT, 128, m_tok], F32R, isOutput=False)
    xtok_d = nc.declare_dram_parameter("xtok", [m_tok, D], F32, isOutput=False)
    wt_d = nc.declare_dram_parameter("wt", [NE, 2, KT, 128, FH], BF16, isOutput=False)
    wg_d = nc.declare_dram_parameter("wg", [128, KT * 128], F32R, isOutput=False)
    ball_d = nc.declare_dram_parameter("ball", [128, D], BF16, isOutput=False)
    mask_d = nc.declare_dram_parameter("mask", [128, IT], F32, isOutput=False)
    if not trivial_affine:
        gam_d = nc.declare_dram_parameter("gam", [128, D], F32, isOutput=False)
        bet_d = nc.declare_dram_parameter("bet", [128, D], F32, isOutput=False)
    out_d = nc.declare_dram_parameter("out", [m_tok, D], F32, isOutput=True)

    with tile.TileContext(nc) as tc, ExitStack() as ctx:
        const = ctx.enter_context(tc.tile_pool(name="const", bufs=1))
        xtp = ctx.enter_context(tc.tile_pool(name="xt", bufs=KT))
        xbp = ctx.enter_context(tc.tile_pool(name="xb", bufs=KT))
        wpool = ctx.enter_context(tc.tile_pool(name="w", bufs=16))
        selp = ctx.enter_context(tc.tile_pool(name="sel", bufs=2 * IT))
        xtokp = ctx.enter_context(tc.tile_pool(name="xtok", bufs=4))
        outp = ctx.enter_context(tc.tile_pool(name="outp", bufs=2))
        scrp = ctx.enter_context(tc.tile_pool(name="scr", bufs=4))
        gatep = ctx.enter_context(tc.tile_pool(name="gate", bufs=IT))
        gbp = ctx.enter_context(tc.tile_pool(name="gb", bufs=IT))
        smallp = ctx.enter_context(tc.tile_pool(name="small", bufs=40))
        zpool = ctx.enter_context(tc.tile_pool(name="z", bufs=4, space="PSUM"))
        bppool = ctx.enter_context(tc.tile_pool(name="bp", bufs=2, space="PSUM"))
        pspool = ctx.enter_context(tc.tile_pool(name="ps", bufs=2, space="PSUM"))

        # ---- PE warm-up source (bf16 zeros; dummies use 256-row streams) ----
        zsrc0 = const.tile([128, FH], F32, tag="zsrc0")
        nc.gpsimd.memset(zsrc0[:], 0.0)
        zsrcb = const.tile([128, FH], BF16, tag="zsrcb")
        nc.vector.tensor_copy(zsrcb[:], zsrc0[:])

        dummy_state = {"n": 0}

        def dummies(n):
            """n cheap bf16 PE filler matmuls (zero @ zero) to hold p-state."""
            dt_ = zpool.tile([128, 256], F32, tag="z", name=f"dps{dummy_state['n']}")
            dummy_state["n"] += 1
            for j in range(n):
                nc.tensor.matmul(
                    dt_[:], zsrcb[:, 0:128], zsrcb[:, 0:256],
                    start=(j == 0), stop=(j == n - 1),
                )

        # ---- small constant inputs ----
        wg_sb = const.tile([128, KT * 128], F32R, tag="wg")
        nc.sync.dma_start(wg_sb[:], wg_d[:])
        ball_sb = const.tile([128, D], BF16, tag="ball")
        nc.sync.dma_start(ball_sb[:], ball_d[:])
        mask_sb = const.tile([128, IT], F32, tag="mask")
        nc.sync.dma_start(mask_sb[:], mask_d[:])
        if not trivial_affine:
            gam_sb = const.tile([128, D], F32, tag="gam")
            nc.sync.dma_start(gam_sb[:], gam_d[:])
            bet_sb = const.tile([128, D], F32, tag="bet")
            nc.sync.dma_start(bet_sb[:], bet_d[:])

        # ---- resident xT k-tiles (f32 for gates), then streamed bf16 weights ----
        xT = []
        for k in range(KT):
            t = xtp.tile([128, m_tok], F32R, tag="xt")
            nc.sync.dma_start(t[:], xt_d[k])
            xT.append(t)
        w_sb = {}
        for e in range(NE):
            for c in (0, 1):
                for k in range(KT):
                    t = wpool.tile([128, FH], BF16, tag="w", name=f"w{e}{c}{k}")
                    nc.sync.dma_start(t[:], wt_d[e, c, k])
                    w_sb[(e, c, k)] = t

        identity = const.tile([128, 128], F32, tag="ident")
        masks.make_identity(nc, identity[:])
        identR = const.tile([128, 128], F32R, tag="identR")
        nc.vector.tensor_copy(identR[:], identity[:])
        identB = const.tile([128, 128], BF16, tag="identB")
        nc.vector.tensor_copy(identB[:], identity[:])

        # ---- bf16 copy of xT for the expert matmuls (DVE/ACT split) ----
        xB = []
        for k in range(KT):
            t = xbp.tile([128, m_tok], BF16, tag="xb")
            if k % 2 == 0:
                nc.vector.tensor_copy(t[:], xT[k][:])
            else:
                nc.scalar.copy(t[:], xT[k][:])
            xB.append(t)

        # ---- accumulators (ping-pong; in-place DVE ops fault) ----
        selA = [selp.tile([128, D], F32, tag="sel", name=f"selA{i}") for i in range(IT)]
        selB = [selp.tile([128, D], F32, tag="sel", name=f"selB{i}") for i in range(IT)]

        # ---- gate logits glT[4, tok], k-paced by the xT DMAs ----
        dummies(10)
        glT_ps = {}
        for h in (0, 1):
            glT_ps[h] = pspool.tile([128, H], F32, tag="ps", name=f"glTps{h}")
        for k in range(KT):
            for h in (0, 1):
                nc.tensor.matmul(
                    glT_ps[h][:], wg_sb[:, k * 128:(k + 1) * 128],
                    xT[k][:, h * H:(h + 1) * H],
                    start=(k == 0), stop=(k == KT - 1),
                )
            if k < KT - 1:
                dummies(2)
        glT_sb = const.tile([NE, m_tok], F32R, tag="glT")
        for h in (0, 1):
            nc.vector.tensor_copy(glT_sb[:, h * H:(h + 1) * H], glT_ps[h][0:NE, :])

        # ---- per token tile: logits transpose (PE), masked softmax, gates
        #      transpose, bias-combine matmuls into the accumulator ----
        glp_t = []
        for i in range(IT):
            glp = pspool.tile([128, NE], F32, tag="ps", name=f"glp{i}")
            nc.tensor.matmul(
                glp[:], glT_sb[:, i * 128:(i + 1) * 128], identR[0:NE, 0:NE],
                start=True, stop=True,
            )
            glp_t.append(glp)
            dummies(2)
        gates_t = []
        gatesb_t = []
        for i in range(IT):
            glp = glp_t[i]
            negmax = smallp.tile([128, 1], F32, tag="s1")
            nc.vector.tensor_reduce(
                negmax[:], glp[:], axis=AX.X, op=ALU.max, negate=True
            )
            exps = smallp.tile([128, NE], F32, tag="s4")
            expsum = smallp.tile([128, 1], F32, tag="s1")
            nc.scalar.activation(
                exps[:], glp[:], ACTF.Exp,
                bias=negmax[:], scale=1.0, accum_out=expsum[:],
            )
            rinv = smallp.tile([128, 1], F32, tag="s1")
            nc.vector.reciprocal(rinv[:], expsum[:])
            rm = smallp.tile([128, 1], F32, tag="s1")
            nc.vector.tensor_mul(rm[:], rinv[:], mask_sb[:, i:i + 1])
            gates = gatep.tile([128, NE], F32, tag="g")
            nc.vector.tensor_scalar_mul(gates[:], exps[:], rm[:])
            gates_t.append(gates)
            gb = gbp.tile([128, NE], BF16, tag="gb")
            nc.vector.tensor_copy(gb[:], gates[:])
            gatesb_t.append(gb)
        dummies(4)
        gTp = const.tile([128, 128], BF16, tag="gTp")
        nc.vector.tensor_copy(gTp[:], zsrcb[:, 0:128])  # rows 4+ stay zero
        for i in range(IT):
            gtp = pspool.tile([NE, 128], F32, tag="ps", name=f"gtp{i}")
            nc.tensor.matmul(
                gtp[:], gatesb_t[i][:], identB[:], start=True, stop=True
            )
            nc.vector.tensor_copy(gTp[0:NE, :], gtp[:])
            for c in (0, 1):
                bp = bppool.tile([128, FH], F32, tag="bp", name=f"bps{i}{c}")
                nc.tensor.matmul(
                    bp[:], gTp[:], ball_sb[:, c * FH:(c + 1) * FH],
                    start=True, stop=True,
                )
                nc.scalar.copy(selA[i][:, c * FH:(c + 1) * FH], bp[:])
            dummies(2)
        dummies(4)

        # ---- expert matmul stream: e outer, feature half c, token tile i.
        #      DVE scalar_tensor_tensor accumulates gate-weighted outputs in
        #      ping-pong SBUF tiles; LN + residual tail inlined in the e3
        #      passes, split across DVE/ACT/GpSimd. ----
        bn6s = [None] * IT
        xi_t = [None] * IT
        src_l, dst_l = selA, selB
        for e in range(NE):
            last = e == NE - 1
            for c in (0, 1):
                cs = slice(c * FH, (c + 1) * FH)
                for i in range(IT):
                    isl = slice(i * 128, (i + 1) * 128)
                    zt = zpool.tile([128, FH], F32, tag="z")
                    for k in range(KT):
                        nc.tensor.matmul(
                            zt[:], xB[k][:, isl], w_sb[(e, c, k)][:],
                            start=(k == 0), stop=(k == KT - 1),
                        )
                    nc.vector.scalar_tensor_tensor(
                        dst_l[i][:, cs], zt[:], gates_t[i][:, e:e + 1],
                        src_l[i][:, cs], op0=ALU.mult, op1=ALU.add,
                    )
                    if not last:
                        continue
                    if c == 0:
                        # prefetch residual x for this tile; stats on half 0
                        xi = xtokp.tile([128, D], F32, tag="xtok")
                        nc.sync.dma_start(xi[:], xtok_d[i * 128:(i + 1) * 128, :])
                        xi_t[i] = xi
                        bn6 = smallp.tile([128, 2 * 6], F32, tag="bn6")
                        nc.vector.bn_stats(bn6[:, 0:6], dst_l[i][:, 0:FH])
                        bn6s[i] = bn6
                        continue
                    selF = dst_l[i]
                    xi = xi_t[i]
                    bn6 = bn6s[i]
                    nc.vector.bn_stats(bn6[:, 6:12], selF[:, FH:D])
                    mv = smallp.tile([128, 2], F32, tag="mv")
                    nc.vector.bn_aggr(mv[:], bn6[:])
                    avi = smallp.tile([128, 1], F32, tag="s1")
                    nc.vector.tensor_scalar_add(avi[:], mv[:, 1:2], EPS)
                    sdi = smallp.tile([128, 1], F32, tag="s1")
                    nc.scalar.sqrt(sdi[:], avi[:])
                    ri = smallp.tile([128, 1], F32, tag="s1")
                    nc.vector.reciprocal(ri[:], sdi[:])
                    mbi = smallp.tile([128, 1], F32, tag="s1")
                    nc.vector.tensor_scalar(
                        mbi[:], mv[:, 0:1], ri[:], -1.0,
                        op0=ALU.mult, op1=ALU.mult,
                    )
                    outt = outp.tile([128, D], F32, tag="out")
                    if trivial_affine:
                        # out = sel*r + (x - mu*r); half 0 DVE, half 1 GpSimd
                        xadj0 = scrp.tile([128, FH], F32, tag="scr")
                        nc.scalar.activation(
                            xadj0[:], xi[:, 0:FH], ACTF.Identity,
                            bias=mbi[:], scale=1.0,
                        )
                        nc.vector.scalar_tensor_tensor(
                            outt[:, 0:FH], selF[:, 0:FH], ri[:], xadj0[:],
                            op0=ALU.mult, op1=ALU.add,
                        )
                        lnb1 = scrp.tile([128, FH], F32, tag="scr")
                        nc.scalar.activation(
                            lnb1[:], selF[:, FH:D], ACTF.Identity,
                            bias=mbi[:], scale=ri[:],
                        )
                        nc.gpsimd.tensor_add(outt[:, FH:D], lnb1[:], xi[:, FH:D])
                    else:
                        lnb = scrp.tile([128, D], F32, tag="scr2")
                        nc.scalar.activation(
                            lnb[:], selF[:], ACTF.Identity,
                            bias=mbi[:], scale=ri[:],
                        )
                        lng = scrp.tile([128, D], F32, tag="scr2")
                        nc.vector.tensor_mul(lng[:], lnb[:], gam_sb[:])
                        lnb2 = scrp.tile([128, D], F32, tag="scr2")
                        nc.vector.tensor_add(lnb2[:], lng[:], bet_sb[:])
                        nc.gpsimd.tensor_add(outt[:, 0:FH], lnb2[:, 0:FH], xi[:, 0:FH])
                        nc.gpsimd.tensor_add(outt[:, FH:D], lnb2[:, FH:D], xi[:, FH:D])
                    nc.sync.dma_start(out_d[i * 128:(i + 1) * 128, :], outt[:])
            src_l, dst_l = dst_l, src_l

    nc.finalize()
    return nc


_PROGRAM_CACHE: dict = {}


def _get_program(m_tok: int, trivial_affine: bool) -> bass.Bass:
    key = (m_tok, trivial_affine)
    if key not in _PROGRAM_CACHE:
        _PROGRAM_CACHE[key] = _build_program(m_tok, trivial_affine)
    return _PROGRAM_CACHE[key]


def _pack_tokens(b: np.ndarray, m_tok: int, use_all_fill: bool):
    """Partition tokens into 8 chunks of m_tok, each chunk holding tokens of a
    single behavior (1..4) plus masked b==0 filler. Returns (cores, leftover)
    where leftover are b==0 tokens not placed on any core (None on failure)."""
    idx0 = np.flatnonzero(b == 0)
    chunks = []
    for t in range(1, NB + 1):
        idxs = np.flatnonzero(b == t)
        for s in range(0, max(len(idxs), 1), m_tok):
            part = idxs[s:s + m_tok]
            if len(part):
                chunks.append((part, t))
    if len(chunks) > NCORES:
        return None, None
    while len(chunks) < NCORES:
        chunks.append((np.empty((0,), np.int64), 1))
    need_total = sum(m_tok - len(p) for p, _ in chunks)
    if need_total > len(idx0):
        return None, None
    p0 = 0
    cores = []
    for part, t in chunks:
        need = m_tok - len(part)
        fill = idx0[p0:p0 + need]
        p0 += need
        idx = np.concatenate([part.astype(np.int64), fill.astype(np.int64)])
        msk = np.zeros((m_tok,), np.float32)
        msk[:len(part)] = 1.0
        cores.append((idx, msk, t))
    leftover = idx0[p0:]
    if use_all_fill and len(leftover):
        return None, None
    return cores, leftover


def _behavior_tensors(W_sh, b_sh, W_sp, b_sp, w_gates):
    per_t = {}
    W_sh_flat = W_sh.reshape(NESH * D, D)
    for t in range(1, NB + 1):
        Wall = np.concatenate([W_sh_flat, W_sp[t - 1:t].reshape(D, D)], axis=0)
        wT = np.ascontiguousarray(Wall.T)                      # [D, NE*D]
        wt_h = np.ascontiguousarray(
            wT.reshape(KT, 128, NE, 2, FH).transpose(2, 3, 0, 1, 4).astype(BF)
        )                                                      # [e, c, k, 128, FH]
        wg_h = np.zeros((128, KT * 128), np.float32)
        wg_k = w_gates[t - 1].reshape(KT, 128, NE).transpose(1, 0, 2)  # [128, KT, NE]
        for k in range(KT):
            wg_h[:, k * 128:k * 128 + NE] = wg_k[:, k, :]
        ball_h = np.zeros((128, D), np.float32)
        ball_h[0:NE] = np.stack([b_sh[0], b_sh[1], b_sh[2], b_sp[t - 1]], axis=0)
        per_t[t] = (wt_h, wg_h, np.ascontiguousarray(ball_h.astype(BF)))
    return per_t


def _prepare(x, b_seq, W_sh, b_sh, W_sp, b_sp, w_gates, gamma, beta):
    x = np.ascontiguousarray(np.asarray(x, dtype=np.float32))
    b = np.asarray(b_seq).astype(np.int64).ravel()
    W_sh = np.asarray(W_sh, dtype=np.float32)
    b_sh = np.asarray(b_sh, dtype=np.float32)
    W_sp = np.asarray(W_sp, dtype=np.float32)
    b_sp = np.asarray(b_sp, dtype=np.float32)
    w_gates = np.asarray(w_gates, dtype=np.float32)
    gamma = np.asarray(gamma, dtype=np.float32)
    beta = np.asarray(beta, dtype=np.float32)
    assert x.shape == (N, D) and b.shape == (N,)

    trivial = bool(np.all(gamma == 1.0) and np.all(beta == 0.0))

    m_tok = M_SKIP
    cores, leftover = _pack_tokens(b, M_SKIP, use_all_fill=False)
    if cores is None:
        m_tok = M_FULL
        cores, leftover = _pack_tokens(b, M_FULL, use_all_fill=False)
        if cores is None:
            raise RuntimeError("token packing failed for both chunk sizes")

    per_t = _behavior_tensors(W_sh, b_sh, W_sp, b_sp, w_gates)

    IT = m_tok // 128
    in_maps = []
    for idx, msk, t in cores:
        wt_h, wg_h, ball_h = per_t[t]
        xc = np.ascontiguousarray(x[idx])                      # [M, D]
        xt_h = np.ascontiguousarray(xc.T).reshape(KT, 128, m_tok)
        m = {
            "xt": xt_h,
            "xtok": xc,
            "wt": wt_h,
            "wg": wg_h,
            "ball": ball_h,
            "mask": np.ascontiguousarray(msk.reshape(IT, 128).T),
        }
        if not trivial:
            m["gam"] = np.ascontiguousarray(np.broadcast_to(gamma, (128, D)))
            m["bet"] = np.ascontiguousarray(np.broadcast_to(beta, (128, D)))
        in_maps.append(m)
    return trivial, m_tok, cores, leftover, (x, beta), in_maps


def kernel_with_results(trace: bool = False, **inputs):
    trivial, m_tok, cores, leftover, (x, beta), in_maps = _prepare(**inputs)
    nc = _get_program(m_tok, trivial)
    res = run_bass_kernel_spmd(
        nc, in_maps, list(range(NCORES)), trace=trace
    )
    out = np.empty((N, D), np.float32)
    for c, (idx, _msk, _t) in enumerate(cores):
        out[idx] = res.results[c]["out"]
    if leftover is not None and len(leftover):
        # b==0 tokens that were not needed as filler: out = x + beta
        out[leftover] = x[leftover] + beta[None, :]
    return out, res


def kernel(**inputs) -> np.ndarray:
    out, _ = kernel_with_results(trace=False, **inputs)
    return out
